# revision 11
# baseline (speedup 1.0000x reference)
"""Trainium2 Bass kernel for nn_Block_38053410242840 (dense transformer block).

Strategy: data-parallel over batch (B=8 -> 8 NeuronCores, zero collectives).
Per core, one batch element [T=1024, C=1024] flows feature-major
(activations stored [feature partitions, token free]) so every matmul's
contraction dim sits on SBUF partitions with no on-device transposes:
the host pre-transposes weights/x and pre-casts weights to bf16.

v2 pipeline (vs v1): x held resident in SBUF; LN uses a single Rsqrt and a
bf16 (rstd | mean*rstd) broadcast; attention runs ch-major (all heads'
first token half, then second) with softmax normalization deferred by one
block and 1/Z computed on DVE (reciprocal_approx_fast) so the ACT engine
runs a pure exp stream with no table reloads; proj's first token half is
emitted between attention blocks to fill the ACT-bound PE gaps; LN2/fc1
run per token half so fc1 overlaps LN2; PSUM pools are scoped per phase.
"""
import sys

sys.path.insert(0, "/opt/trn_rl_repo")

from contextlib import ExitStack

import ml_dtypes
import numpy as np

import concourse.bass as bass
import concourse.tile as tile
from concourse import bacc, mybir
from concourse import bass_utils

F32 = mybir.dt.float32
BF16 = mybir.dt.bfloat16
AF = mybir.ActivationFunctionType
ALU = mybir.AluOpType
ts = bass.ts

P = 128
T = 1024
C = 1024
H = 16
HD = 64
LN_EPS = 1e-5
NB = 8  # cores / batch


def act_raw(nc, out, in_, func, bias=0.0, scale=1.0):
    """InstActivation with immediate bias/scale (bypasses accuracy guards)."""
    eng = nc.scalar
    inputs = [eng.lower_ap(in_)]
    for arg in (bias, scale, 0.0):
        inputs.append(mybir.ImmediateValue(dtype=mybir.dt.float32, value=arg))
    return eng.add_instruction(
        mybir.InstActivation(
            name=nc.get_next_instruction_name(),
            func=func,
            ins=inputs,
            outs=[eng.lower_ap(out)],
        )
    )


def build_nc(debug=False):
    nc = bacc.Bacc("TRN2", target_bir_lowering=False, debug=False,
                   enable_asserts=False, num_devices=NB)

    d_xT = nc.dram_tensor("xT", [C, T], F32, kind="ExternalInput").ap()
    d_wkq = nc.dram_tensor("wkq", [C, 2048], BF16, kind="ExternalInput").ap()
    d_wv = nc.dram_tensor("wv", [C, 1024], BF16, kind="ExternalInput").ap()
    d_pw = nc.dram_tensor("pw", [C, 1024], BF16, kind="ExternalInput").ap()
    d_w1 = nc.dram_tensor("w1", [C, 4096], BF16, kind="ExternalInput").ap()
    d_w2 = nc.dram_tensor("w2", [4096, 1024], BF16, kind="ExternalInput").ap()
    # packed f32 consts: [:,0:128]=ones, 128:144 kq bias, 144:152 proj bias(+pb),
    # 152:184 fc1 bias, 184:192 fc2 bias
    d_cfb = nc.dram_tensor("cfb", [P, 200], F32, kind="ExternalInput").ap()
    # packed bf16 consts: [:,0:1024]=causal diag-block masks (2x512), col 1023 ones
    d_mo = nc.dram_tensor("mo", [P, 1024], BF16, kind="ExternalInput").ap()
    d_out = nc.dram_tensor("out", [C, T], F32, kind="ExternalOutput").ap()

    dbg = {}
    if debug:
        dbg["h1"] = nc.dram_tensor("dbg_h1", [P, 8192], BF16, kind="ExternalOutput").ap()
        dbg["k"] = nc.dram_tensor("dbg_k", [P, 8192], BF16, kind="ExternalOutput").ap()
        dbg["q"] = nc.dram_tensor("dbg_q", [P, 8192], BF16, kind="ExternalOutput").ap()
        dbg["v"] = nc.dram_tensor("dbg_v", [P, 8320], BF16, kind="ExternalOutput").ap()
        dbg["y"] = nc.dram_tensor("dbg_y", [P, 8192], BF16, kind="ExternalOutput").ap()
        dbg["x2"] = nc.dram_tensor("dbg_x2", [P, 8192], F32, kind="ExternalOutput").ap()
        dbg["g"] = nc.dram_tensor("dbg_g", [P, 32768], BF16, kind="ExternalOutput").ap()

    with tile.TileContext(nc) as tc:
        with ExitStack() as ctx:
            build_body(ctx, tc, nc, d_xT, d_wkq, d_wv, d_pw, d_w1, d_w2,
                       d_cfb, d_mo, d_out, dbg)
    nc.compile()
    return nc


def build_body(ctx, tc, nc, d_xT, d_wkq, d_wv, d_pw, d_w1, d_w2, d_cfb, d_mo,
               d_out, dbg):
    wp = ctx.enter_context(tc.tile_pool(name="wp", bufs=2))
    lnp = ctx.enter_context(tc.tile_pool(name="lnp", bufs=2))
    tmpp = ctx.enter_context(tc.tile_pool(name="tmpp", bufs=2))
    outp = ctx.enter_context(tc.tile_pool(name="outp", bufs=2))

    # ---- constants ----
    cfb, free_cfb = tc.tile([P, 200], F32, name="cfb_t")
    nc.sync.dma_start(cfb[:], d_cfb[:])
    mo, free_mo = tc.tile([P, 1024], BF16, name="mo_t")
    nc.sync.dma_start(mo[:], d_mo[:])
    ones_f = cfb[:, 0:128]
    kqb = cfb[:, 128:144]
    pbc = cfb[:, 144:152]
    b1c = cfb[:, 152:184]
    b2c = cfb[:, 184:192]
    zero_c = cfb[:, 193:194]
    masks = mo[:, 0:1024]  # diagonal-block mask, doubled
    ones_b = mo[:, 1023:1024]  # all-ones column (bf16)


    def layernorm(get_src, put_dst, ntok, psS_pool, pb_pool, hname):
        """get_src(i) -> [128, ntok] f32 chunk i; put_dst(i) -> [128, ntok]
        bf16 dest. LN stats via ones-matmuls over partitions; rstd via one
        Rsqrt; broadcast (rstd | mean*rstd) in bf16; h = x*rstd - mrs."""
        ncol = ntok // 512
        psS = [psS_pool.tile([33, 512], F32, tag="s", name=f"psS_{hname}")
               for _ in range(ncol)]
        for i in range(8):
            xbsq = lnp.tile([P, 2 * ntok], BF16, tag="xbsq", name="xbsq")
            xb = xbsq[:, 0:ntok]
            sq = xbsq[:, ntok:2 * ntok]
            nc.vector.tensor_copy(xb, get_src(i))
            nc.vector.tensor_mul(sq, xb, xb)
            for c in range(ncol):
                nc.tensor.matmul(psS[c][0:1, :], lhsT=ones_b[:, 0:1],
                                 rhs=xb[:, ts(c, 512)], start=(i == 0), stop=(i == 7))
                nc.tensor.matmul(psS[c][32:33, :], lhsT=ones_b[:, 0:1],
                                 rhs=sq[:, ts(c, 512)], start=(i == 0), stop=(i == 7),
                                 tile_position=(0, 32))
        # stats rows, all on partition 0 (DVE two-SBUF-input ops require
        # equal base partitions): mean | E[x2] -> var | rstd | mean*rstd
        bc, free_bc = tc.tile([P, 2 * ntok], BF16, name=hname + "_bc")
        rows, free_rows = tc.tile([1, 4 * ntok], F32, name=hname + "_rows")
        for c in range(ncol):
            nc.vector.tensor_scalar_mul(rows[0:1, ts(c, 512)], psS[c][0:1, :], 1.0 / C)
            nc.vector.tensor_scalar_mul(rows[0:1, ntok + 512 * c:ntok + 512 * c + 512],
                                        psS[c][32:33, :], 1.0 / C)
        nc.vector.tensor_mul(rows[0:1, 2 * ntok:3 * ntok], rows[0:1, 0:ntok],
                             rows[0:1, 0:ntok])
        nc.vector.tensor_sub(rows[0:1, ntok:2 * ntok], rows[0:1, ntok:2 * ntok],
                             rows[0:1, 2 * ntok:3 * ntok])
        act_raw(nc, rows[0:1, 2 * ntok:3 * ntok], rows[0:1, ntok:2 * ntok], AF.Rsqrt,
                bias=LN_EPS)
        nc.vector.tensor_mul(rows[0:1, 3 * ntok:4 * ntok], rows[0:1, 0:ntok],
                             rows[0:1, 2 * ntok:3 * ntok])
        for c in range(ncol):
            pbt = pb_pool.tile([P, 1024], F32, tag="a", name=f"pb_{hname}")
            nc.tensor.matmul(pbt[:, 0:512], lhsT=ones_f[0:1, 0:128],
                             rhs=rows[0:1, 2 * ntok + 512 * c:2 * ntok + 512 * c + 512],
                             start=True, stop=True)
            nc.tensor.matmul(pbt[:, 512:1024], lhsT=ones_f[0:1, 0:128],
                             rhs=rows[0:1, 3 * ntok + 512 * c:3 * ntok + 512 * c + 512],
                             start=True, stop=True)
            nc.scalar.copy(bc[:, 512 * c:512 * c + 512], pbt[:, 0:512])
            nc.scalar.copy(bc[:, ntok + 512 * c:ntok + 512 * c + 512],
                           pbt[:, 512:1024])
        free_rows()
        for i in range(8):
            t1 = tmpp.tile([P, ntok], BF16, tag="t1", name="t1")
            nc.vector.tensor_mul(t1[:], get_src(i), bc[:, 0:ntok])
            nc.vector.tensor_sub(put_dst(i), t1[:], bc[:, ntok:2 * ntok])
        free_bc()

    # stack-ordered allocation (frees are LIFO): long-lived at the bottom
    x2_all, free_x2 = tc.tile([P, 8192], F32, name="x2_all")
    y2_all, free_y2 = tc.tile([P, 8192], BF16, name="y2_all")
    k_all, free_k = tc.tile([P, 8192], BF16, name="k_all")
    q_all, free_q = tc.tile([P, 8192], BF16, name="q_all")
    v_all, free_v = tc.tile([P, 8320], BF16, name="v_all")
    h1, free_h1 = tc.tile([P, 8192], BF16, name="h1")

    # ---- x resident in SBUF during LN1 (feature-major c-chunks) ----
    x_all, free_x = tc.tile([P, 8192], F32, name="x_all")
    for i in range(8):
        nc.sync.dma_start(x_all[:, ts(i, 1024)], d_xT[ts(i, 128), :])

    # ================= phase 1: LN1 + kq + v =================
    with tc.tile_pool(name="pa1", bufs=3, space="PSUM") as pa1, \
         tc.tile_pool(name="psS1", bufs=2, space="PSUM") as psS1:
        layernorm(lambda i: x_all[:, ts(i, 1024)],
                  lambda i: h1[:, ts(i, 1024)], 1024, psS1, pa1, "h1")
        free_x()
        if dbg:
            nc.sync.dma_start(dbg["h1"][:], h1[:])
        # preload the exp ACT table while ACT is idle (off critical path)
        scr, free_scr = tc.tile([1, 8], F32, name="scr")
        act_raw(nc, scr[0:1, 0:1], cfb[0:1, 193:194], AF.Exp)

        # ---- kq projections: per head, psum [128 = k(64)+q(64), 1024t] ----
        wv_all, free_wv = tc.tile([P, 8192], BF16, name="wv_all")
        wkq_v = d_wkq.rearrange("(ct p) o -> p ct o", p=128)
        for g4 in range(4):
            wg = wp.tile([P, 4096], BF16, tag="wg", name="wg")
            wgv = wg[:].rearrange("p (ct o) -> p ct o", o=512)
            for cc in range(4):
                nc.sync.dma_start(wgv[:, 2 * cc:2 * cc + 2, :],
                                  wkq_v[:, 2 * cc:2 * cc + 2, ts(g4, 512)])
            for hl in range(4):
                hh = 4 * g4 + hl
                j, r = hh // 2, (hh % 2) * 64
                pp = pa1.tile([P, 1024], F32, tag="a", name="ppkq")
                for c in range(8):
                    for ch in range(2):
                        nc.tensor.matmul(pp[:, ts(ch, 512)], lhsT=wgv[:, c, ts(hl, 128)],
                                         rhs=h1[:, 1024 * c + 512 * ch:1024 * c + 512 * ch + 512],
                                         start=(c == 0), stop=(c == 7))
                col = 1024 * j
                nc.scalar.activation(k_all[r:r + 64, col:col + 1024], pp[0:64, :],
                                     AF.Identity, bias=kqb[0:64, hh:hh + 1])
                nc.scalar.activation(q_all[r:r + 64, col:col + 1024], pp[64:128, :],
                                     AF.Identity, bias=kqb[64:128, hh:hh + 1])

        # ---- v projection (token-major, fused ones column per head) ----
        wv_v = d_wv.rearrange("(ct p) o -> p ct o", p=128)
        for c in range(8):
            nc.sync.dma_start(wv_all[:, ts(c, 1024)], wv_v[:, c, :])
        v_view = v_all[:].rearrange("p (a c) -> p a c", c=65)
        nc.vector.memset(v_view[:, :, 64:65], 1.0)
        v_hview = v_all[:].rearrange("p (jt h c) -> p jt h c", jt=8, c=65)
        for jt in range(8):
            psv = pa1.tile([P, 1024], F32, tag="a", name="psv")
            for c in range(8):
                lhs = h1[:, 1024 * c + 128 * jt:1024 * c + 128 * jt + 128]
                for half in range(2):
                    nc.tensor.matmul(psv[:, ts(half, 512)], lhsT=lhs,
                                     rhs=wv_all[:, 1024 * c + 512 * half:1024 * c + 512 * half + 512],
                                     start=(c == 0), stop=(c == 7))
            nc.scalar.copy(v_hview[:, jt, :, 0:64],
                           psv[:].rearrange("p (h c) -> p h c", c=64))
        free_wv()
        free_scr()
        free_h1()
    if dbg:
        nc.sync.dma_start(dbg["k"][:], k_all[:])
        nc.sync.dma_start(dbg["q"][:], q_all[:])
        nc.sync.dma_start(dbg["v"][:], v_all[:])

    # ================= phase 2: attention (ch-major) + proj half 0 =========
    # Per block (j, ch): scores pair (row-group concurrent) -> exp (pure ACT
    # stream) -> diag mask (DVE) -> av accumulate [y'(64); Z] in [65,512]
    # psum pairs. y' evicted to y_all (DVE), Z rows to zpair (GpSimd).
    # Normalization (1/Z on DVE via reciprocal_approx_fast, broadcast via
    # K=1 matmuls, multiply on DVE) is deferred by one block so nothing
    # stalls. proj token-half-0 matmuls are emitted between ch=1 blocks to
    # keep the PE busy while ACT chews the exp backlog.
    # per-block y'+Z eviction target: [65,512] f32 slots (4 = parity x head),
    # y' dims at partitions 0:64, softmax denominator Z at partition 64.
    ybuf, free_y = tc.tile([65, 2048], F32, name="ybuf")
    e_buf, free_e = tc.tile([P, 4096], BF16, name="e_buf")
    e_rot = [0]

    def e_slot():
        i = e_rot[0] % 4
        e_rot[0] += 1
        return e_buf[:, 1024 * i:1024 * i + 1024]

    pw_v = d_pw.rearrange("(ct p) o -> p ct o", p=128)
    proj_wgs = {}

    def load_proj_wg(jg, nm):
        wg = wp.tile([P, 4096], BF16, tag="wg", name=nm)
        wgv = wg[:].rearrange("p (ct o) -> p ct o", o=512)
        for cc in range(4):
            nc.sync.dma_start(wgv[:, 2 * cc:2 * cc + 2, :],
                              pw_v[:, 2 * cc:2 * cc + 2, ts(jg, 512)])
        proj_wgs[jg] = wgv

    def proj_piece(jj, h, pool):
        wgv = proj_wgs[jj // 4]
        pp = pool.tile([P, 512], F32, tag="pj", name="ppj")
        for c in range(8):
            nc.tensor.matmul(pp[:], lhsT=wgv[:, c, ts(jj % 4, 128)],
                             rhs=y2_all[:, 1024 * c + 512 * h:1024 * c + 512 * h + 512],
                             start=(c == 0), stop=(c == 7))
        xr = tmpp.tile([P, 512], F32, tag="xr", name="xr")
        nc.sync.dma_start(xr[:], d_xT[ts(jj, 128), 512 * h:512 * h + 512])
        nc.vector.scalar_tensor_tensor(
            x2_all[:, 1024 * jj + 512 * h:1024 * jj + 512 * h + 512],
            pp[:], pbc[:, jj:jj + 1], xr[:],
            ALU.add, ALU.add)

    # ping-pong invZ rows (partition 64, matching ybuf's Z row)
    zibuf, free_zibuf = tc.tile([65, 2048], F32, name="zibuf")
    zrot = [0]

    with tc.tile_pool(name="psA", bufs=2, space="PSUM") as psA, \
         tc.tile_pool(name="psB", bufs=3, space="PSUM") as psB, \
         tc.tile_pool(name="pph0", bufs=1, space="PSUM") as pph0:
        pend = [None]

        def emit_normalize(j, ch, par):
            col = 1024 * j + 512 * ch
            # NB: recip_approx_fast mislowers single-partition APs at a
            # nonzero base (HW-probed); run it over the full [65,1024] view
            # instead — rows 0:64 (1/y', may hit the undefined ±0 edge case)
            # are junk and never read, only the Z row (64) is consumed.
            nc.vector.reciprocal_approx_fast(
                zibuf[:, ts(par, 1024)], ybuf[:, ts(par, 1024)])
            pz = psA.tile([P, 1024], F32, tag="a", name="pz")
            for m2 in range(2):
                nc.tensor.matmul(pz[0:64, ts(m2, 512)],
                                 lhsT=ones_f[64:65, 0:64],
                                 rhs=zibuf[64:65, 1024 * par + 512 * m2:1024 * par + 512 * m2 + 512],
                                 start=True, stop=True)
            for m2 in range(2):
                r = 64 * m2
                nc.vector.tensor_mul(y2_all[r:r + 64, col:col + 512],
                                     pz[0:64, ts(m2, 512)],
                                     ybuf[0:64, ts(2 * par + m2, 512)])

        def emit_block(j, ch):
            ntk = 4 if ch == 0 else 8
            qcol = 1024 * j + 512 * ch
            py = [psB.tile([65, 512], F32, tag="b", name="py") for _ in range(2)]
            for jt in range(ntk):
                pcol = 1024 * j + 128 * jt
                m = jt - 4 * ch
                o = 128 * m if m > 0 else 0
                ps_ = psA.tile([P, 1024], F32, tag="a", name="ps")
                for m2 in range(2):
                    r = 64 * m2
                    nc.tensor.matmul(ps_[:, 512 * m2 + o:512 * m2 + 512],
                                     lhsT=k_all[r:r + 64, pcol:pcol + 128],
                                     rhs=q_all[r:r + 64, qcol + o:qcol + 512],
                                     start=True, stop=True)
                et_t = e_slot()
                et = et_t.rearrange("p (h c) -> p h c", c=512)
                ps_v = ps_[:].rearrange("p (h c) -> p h c", c=512)
                nc.scalar.activation(et[:, :, o:512], ps_v[:, :, o:512],
                                     AF.Exp, bias=zero_c, scale=0.125)
                if m >= 0:
                    nc.vector.tensor_mul(
                        et[:, :, o:o + 128], et[:, :, o:o + 128],
                        masks[:].rearrange("p (h c) -> p h c", c=512)[:, 0:2, 0:128])
                for m2 in range(2):
                    hh = 2 * j + m2
                    nc.tensor.matmul(
                        py[m2][:, o:512],
                        lhsT=v_all[:, 1040 * jt + 65 * hh:1040 * jt + 65 * hh + 65],
                        rhs=et[:, m2, o:512],
                        start=(jt == 0), stop=(jt == ntk - 1))
                # deferred normalize of the previous block, emitted after this
                # block's second av so its pz never stalls the PE queue.
                if jt == 1 and pend[0] is not None:
                    emit_normalize(*pend[0])
                    pend[0] = None
            par = zrot[0] % 2
            zrot[0] += 1
            for m2 in range(2):
                nc.vector.tensor_copy(ybuf[:, ts(2 * par + m2, 512)],
                                      py[m2][:, :])
            pend[0] = (j, ch, par)

        for j in range(8):
            emit_block(j, 0)
            if j == 6:
                load_proj_wg(0, "wgp")
            if j == 7:
                load_proj_wg(1, "wgp2")
        for j in range(8):
            emit_block(j, 1)
            # proj half-0 piece jj=j-1: all its y2 half-0 inputs are final
            # once the deferred ch=0 normalizes have run (done by block (1,1)).
            if j >= 2:
                proj_piece(j - 2, 0, pph0)
        emit_normalize(*pend[0])
        pend[0] = None
        proj_piece(6, 0, pph0)
        proj_piece(7, 0, pph0)
    free_zibuf()
    free_e()
    free_y()
    if dbg:
        nc.sync.dma_start(dbg["y"][:], y2_all[:])

    # ================= phase 3: proj half 1 + LN2 + fc1 + fc2 ==============
    with tc.tile_pool(name="pC", bufs=2, space="PSUM") as pC, \
         tc.tile_pool(name="pD", bufs=2, space="PSUM") as pD, \
         tc.tile_pool(name="psS2", bufs=2, space="PSUM") as psS2:
        load_proj_wg(0, "wgp1b")
        load_proj_wg(1, "wgp2b")
        for jj in range(8):
            proj_piece(jj, 1, pD)
        free_v()
        free_q()
        free_k()
        free_y2()
        if dbg:
            nc.sync.dma_start(dbg["x2"][:], x2_all[:])

        h2, free_h2 = tc.tile([P, 8192], BF16, name="h2")
        g_all, free_g = tc.tile([P, 32768], BF16, name="g_all")
        w1_v = d_w1.rearrange("(ct p) o -> p ct o", p=128)

        def fc1_half(h):
            for og in range(8):
                wg = wp.tile([P, 4096], BF16, tag="wg", name="wg1")
                wgv = wg[:].rearrange("p (ct o) -> p ct o", o=512)
                for cc in range(4):
                    nc.sync.dma_start(wgv[:, 2 * cc:2 * cc + 2, :],
                                      w1_v[:, 2 * cc:2 * cc + 2, ts(og, 512)])
                for ol in range(4):
                    oo = 4 * og + ol
                    pp = pD.tile([P, 512], F32, tag="pj", name="pp1")
                    for c in range(8):
                        nc.tensor.matmul(pp[:], lhsT=wgv[:, c, ts(ol, 128)],
                                         rhs=h2[:, 1024 * c + 512 * h:1024 * c + 512 * h + 512],
                                         start=(c == 0), stop=(c == 7))
                    nc.scalar.activation(
                        g_all[:, 1024 * oo + 512 * h:1024 * oo + 512 * h + 512],
                        pp[:], AF.Gelu, bias=b1c[:, oo:oo + 1])

        for h in range(2):
            layernorm(lambda i: x2_all[:, 1024 * i + 512 * h:1024 * i + 512 * h + 512],
                      lambda i: h2[:, 1024 * i + 512 * h:1024 * i + 512 * h + 512],
                      512, psS2, pC, f"h2_{h}")
            fc1_half(h)
        if dbg:
            nc.sync.dma_start(dbg["g"][:], g_all[:])

        # ---- fc2 + residual -> out ----
        w2_v = d_w2.rearrange("(kk p) o -> p kk o", p=128)
        for j in range(8):
            wg = wp.tile([P, 4096], BF16, tag="wg", name="wg2")
            wgv = wg[:].rearrange("p (kk o) -> p kk o", o=128)
            for kg in range(4):
                nc.sync.dma_start(wgv[:, 8 * kg:8 * kg + 8, :],
                                  w2_v[:, 8 * kg:8 * kg + 8, ts(j, 128)])
            pp = pC.tile([P, 1024], F32, tag="a", name="pp2")
            for kk in range(32):
                for ch in range(2):
                    nc.tensor.matmul(pp[:, ts(ch, 512)], lhsT=wgv[:, kk, :],
                                     rhs=g_all[:, 1024 * kk + 512 * ch:1024 * kk + 512 * ch + 512],
                                     start=(kk == 0), stop=(kk == 31))
            x3 = outp.tile([P, 1024], F32, tag="x3", name="x3")
            nc.vector.scalar_tensor_tensor(
                x3[:], pp[:], b2c[:, j:j + 1],
                x2_all[:, ts(j, 1024)], ALU.add, ALU.add)
            nc.sync.dma_start(d_out[ts(j, 128), :], x3[:])
        free_g()
    free_h2()
    free_x2()
    free_mo()
    free_cfb()


# ---------------- host side ----------------

def prep_inputs(inputs):
    """Build the per-core in_maps from the full problem inputs."""
    f32 = np.float32
    bf16 = ml_dtypes.bfloat16
    x = np.asarray(inputs["x"], f32)
    kqv_w = np.asarray(inputs["kqv_w"], f32)
    kqv_b = np.asarray(inputs["kqv_b"], f32)
    proj_w = np.asarray(inputs["proj_w"], f32)
    proj_b = np.asarray(inputs["proj_b"], f32)
    fc1_w = np.asarray(inputs["fc1_w"], f32)
    fc1_b = np.asarray(inputs["fc1_b"], f32)
    fc2_w = np.asarray(inputs["fc2_w"], f32)
    fc2_b = np.asarray(inputs["fc2_b"], f32)

    wT = np.ascontiguousarray(kqv_w.T).reshape(C, H, 192)
    wkq = np.ascontiguousarray(wT[:, :, :128].reshape(C, 2048)).astype(bf16)
    wv = np.ascontiguousarray(wT[:, :, 128:].reshape(C, 1024)).astype(bf16)
    pw = np.ascontiguousarray(proj_w.T).astype(bf16)
    w1 = np.ascontiguousarray(fc1_w.T).astype(bf16)
    w2 = np.ascontiguousarray(fc2_w.T).astype(bf16)

    kq_b = kqv_b.reshape(H, 192)[:, :128].T  # [128, 16]
    v_b = kqv_b.reshape(H, 192)[:, 128:].reshape(C)
    pb = proj_b + proj_w.astype(np.float64) @ v_b.astype(np.float64)
    pb_col = pb.astype(f32).reshape(8, 128).T  # [128, 8]
    b1_col = fc1_b.reshape(32, 128).T  # [128, 32]
    b2_col = fc2_b.reshape(8, 128).T  # [128, 8]

    cfb = np.zeros((P, 200), f32)
    cfb[:, 0:128] = 1.0
    cfb[:, 128:144] = kq_b
    cfb[:, 144:152] = pb_col
    cfb[:, 152:184] = b1_col
    cfb[:, 184:192] = b2_col
    cfb[:, 192] = LN_EPS

    mo = np.zeros((P, 1024), np.float32)
    pcol = np.arange(128)[:, None]
    frow = np.arange(512)[None, :]
    blk = (frow >= pcol).astype(np.float32)
    mo[:, 0:512] = blk
    mo[:, 512:1024] = blk
    mo = mo.astype(bf16)

    xT = np.ascontiguousarray(x.transpose(0, 2, 1)).astype(f32)  # [B, C, T]

    shared = dict(wkq=wkq, wv=wv, pw=pw, w1=w1, w2=w2, cfb=cfb, mo=mo)
    in_maps = [dict(shared, xT=xT[b]) for b in range(NB)]
    return in_maps


_CACHE = {}


def get_nc(debug=False):
    key = bool(debug)
    if key not in _CACHE:
        _CACHE[key] = build_nc(debug=debug)
    return _CACHE[key]


def run(inputs, debug=False, trace=False):
    nc = get_nc(debug=debug)
    in_maps = prep_inputs(inputs)
    res = bass_utils.run_bass_kernel_spmd(nc, in_maps, core_ids=list(range(NB)),
                                          trace=trace)
    return res


def kernel(**inputs):
    res = run(inputs, debug=False, trace=False)
    out = np.stack([np.asarray(res.results[b]["out"]).T for b in range(NB)])
    return np.ascontiguousarray(out.astype(np.float32))


# revision 13
# speedup vs baseline: 1.0229x; 1.0229x over previous
"""Trainium2 Bass kernel for nn_Block_38053410242840 (dense transformer block).

Strategy: data-parallel over batch (B=8 -> 8 NeuronCores, zero collectives).
Per core, one batch element [T=1024, C=1024] flows feature-major
(activations stored [feature partitions, token free]) so every matmul's
contraction dim sits on SBUF partitions with no on-device transposes:
the host pre-transposes weights/x and pre-casts weights to bf16.

v2 pipeline (vs v1): x held resident in SBUF; LN uses a single Rsqrt and a
bf16 (rstd | mean*rstd) broadcast; attention runs ch-major (all heads'
first token half, then second) with softmax normalization deferred by one
block and 1/Z computed on DVE (reciprocal_approx_fast) so the ACT engine
runs a pure exp stream with no table reloads; proj's first token half is
emitted between attention blocks to fill the ACT-bound PE gaps; LN2/fc1
run per token half so fc1 overlaps LN2; PSUM pools are scoped per phase.
"""
import sys

sys.path.insert(0, "/opt/trn_rl_repo")

from contextlib import ExitStack

import ml_dtypes
import numpy as np

import concourse.bass as bass
import concourse.tile as tile
from concourse import bacc, mybir
from concourse import bass_utils

F32 = mybir.dt.float32
BF16 = mybir.dt.bfloat16
AF = mybir.ActivationFunctionType
ALU = mybir.AluOpType
ts = bass.ts

P = 128
T = 1024
C = 1024
H = 16
HD = 64
LN_EPS = 1e-5
NB = 8  # cores / batch


def act_raw(nc, out, in_, func, bias=0.0, scale=1.0):
    """InstActivation with immediate bias/scale (bypasses accuracy guards)."""
    eng = nc.scalar
    inputs = [eng.lower_ap(in_)]
    for arg in (bias, scale, 0.0):
        inputs.append(mybir.ImmediateValue(dtype=mybir.dt.float32, value=arg))
    return eng.add_instruction(
        mybir.InstActivation(
            name=nc.get_next_instruction_name(),
            func=func,
            ins=inputs,
            outs=[eng.lower_ap(out)],
        )
    )


def build_nc(debug=False):
    nc = bacc.Bacc("TRN2", target_bir_lowering=False, debug=False,
                   enable_asserts=False, num_devices=NB)

    d_xT = nc.dram_tensor("xT", [C, T], F32, kind="ExternalInput").ap()
    d_wkq = nc.dram_tensor("wkq", [C, 2048], BF16, kind="ExternalInput").ap()
    d_wv = nc.dram_tensor("wv", [C, 1024], BF16, kind="ExternalInput").ap()
    d_pw = nc.dram_tensor("pw", [C, 1024], BF16, kind="ExternalInput").ap()
    d_w1 = nc.dram_tensor("w1", [C, 4096], BF16, kind="ExternalInput").ap()
    d_w2 = nc.dram_tensor("w2", [4096, 1024], BF16, kind="ExternalInput").ap()
    # packed f32 consts: [:,0:128]=ones, 128:144 kq bias, 144:152 proj bias(+pb),
    # 152:184 fc1 bias, 184:192 fc2 bias
    d_cfb = nc.dram_tensor("cfb", [P, 200], F32, kind="ExternalInput").ap()
    # packed bf16 consts: [:,0:1024]=causal diag-block masks (2x512), col 1023 ones
    d_mo = nc.dram_tensor("mo", [P, 1024], BF16, kind="ExternalInput").ap()
    d_out = nc.dram_tensor("out", [C, T], F32, kind="ExternalOutput").ap()

    dbg = {}
    if debug:
        dbg["h1"] = nc.dram_tensor("dbg_h1", [P, 8192], BF16, kind="ExternalOutput").ap()
        dbg["k"] = nc.dram_tensor("dbg_k", [P, 8192], BF16, kind="ExternalOutput").ap()
        dbg["q"] = nc.dram_tensor("dbg_q", [P, 8192], BF16, kind="ExternalOutput").ap()
        dbg["v"] = nc.dram_tensor("dbg_v", [P, 8320], BF16, kind="ExternalOutput").ap()
        dbg["y"] = nc.dram_tensor("dbg_y", [P, 8192], BF16, kind="ExternalOutput").ap()
        dbg["x2"] = nc.dram_tensor("dbg_x2", [P, 8192], F32, kind="ExternalOutput").ap()
        dbg["g"] = nc.dram_tensor("dbg_g", [P, 32768], BF16, kind="ExternalOutput").ap()

    with tile.TileContext(nc) as tc:
        with ExitStack() as ctx:
            build_body(ctx, tc, nc, d_xT, d_wkq, d_wv, d_pw, d_w1, d_w2,
                       d_cfb, d_mo, d_out, dbg)
    nc.compile()
    return nc


def build_body(ctx, tc, nc, d_xT, d_wkq, d_wv, d_pw, d_w1, d_w2, d_cfb, d_mo,
               d_out, dbg):
    wp = ctx.enter_context(tc.tile_pool(name="wp", bufs=2))
    lnp = ctx.enter_context(tc.tile_pool(name="lnp", bufs=2))
    tmpp = ctx.enter_context(tc.tile_pool(name="tmpp", bufs=2))
    outp = ctx.enter_context(tc.tile_pool(name="outp", bufs=2))

    # ---- constants ----
    cfb, free_cfb = tc.tile([P, 200], F32, name="cfb_t")
    nc.sync.dma_start(cfb[:], d_cfb[:])
    mo, free_mo = tc.tile([P, 1024], BF16, name="mo_t")
    nc.sync.dma_start(mo[:], d_mo[:])
    ones_f = cfb[:, 0:128]
    kqb = cfb[:, 128:144]
    pbc = cfb[:, 144:152]
    b1c = cfb[:, 152:184]
    b2c = cfb[:, 184:192]
    zero_c = cfb[:, 193:194]
    masks = mo[:, 0:1024]  # diagonal-block mask, doubled
    ones_b = mo[:, 1023:1024]  # all-ones column (bf16)


    def layernorm(get_src, put_dst, ntok, psS_pool, pb_pool, hname):
        """get_src(i) -> [128, ntok] f32 chunk i; put_dst(i) -> [128, ntok]
        bf16 dest. LN stats via ones-matmuls over partitions; rstd via one
        Rsqrt; broadcast (rstd | mean*rstd) in bf16; h = x*rstd - mrs."""
        ncol = ntok // 512
        psS = [psS_pool.tile([33, 512], F32, tag="s", name=f"psS_{hname}")
               for _ in range(ncol)]
        for i in range(8):
            xbsq = lnp.tile([P, 2 * ntok], BF16, tag="xbsq", name="xbsq")
            xb = xbsq[:, 0:ntok]
            sq = xbsq[:, ntok:2 * ntok]
            nc.vector.tensor_copy(xb, get_src(i))
            nc.vector.tensor_mul(sq, xb, xb)
            for c in range(ncol):
                nc.tensor.matmul(psS[c][0:1, :], lhsT=ones_b[:, 0:1],
                                 rhs=xb[:, ts(c, 512)], start=(i == 0), stop=(i == 7))
                nc.tensor.matmul(psS[c][32:33, :], lhsT=ones_b[:, 0:1],
                                 rhs=sq[:, ts(c, 512)], start=(i == 0), stop=(i == 7),
                                 tile_position=(0, 32))
        # stats rows, all on partition 0 (DVE two-SBUF-input ops require
        # equal base partitions): mean | E[x2] -> var | rstd | mean*rstd
        bc, free_bc = tc.tile([P, 2 * ntok], BF16, name=hname + "_bc")
        rows, free_rows = tc.tile([1, 4 * ntok], F32, name=hname + "_rows")
        for c in range(ncol):
            nc.vector.tensor_scalar_mul(rows[0:1, ts(c, 512)], psS[c][0:1, :], 1.0 / C)
            nc.vector.tensor_scalar_mul(rows[0:1, ntok + 512 * c:ntok + 512 * c + 512],
                                        psS[c][32:33, :], 1.0 / C)
        nc.vector.tensor_mul(rows[0:1, 2 * ntok:3 * ntok], rows[0:1, 0:ntok],
                             rows[0:1, 0:ntok])
        nc.vector.tensor_sub(rows[0:1, ntok:2 * ntok], rows[0:1, ntok:2 * ntok],
                             rows[0:1, 2 * ntok:3 * ntok])
        act_raw(nc, rows[0:1, 2 * ntok:3 * ntok], rows[0:1, ntok:2 * ntok], AF.Rsqrt,
                bias=LN_EPS)
        nc.vector.tensor_mul(rows[0:1, 3 * ntok:4 * ntok], rows[0:1, 0:ntok],
                             rows[0:1, 2 * ntok:3 * ntok])
        for c in range(ncol):
            pbt = pb_pool.tile([P, 1024], F32, tag="a", name=f"pb_{hname}")
            nc.tensor.matmul(pbt[:, 0:512], lhsT=ones_f[0:1, 0:128],
                             rhs=rows[0:1, 2 * ntok + 512 * c:2 * ntok + 512 * c + 512],
                             start=True, stop=True)
            nc.tensor.matmul(pbt[:, 512:1024], lhsT=ones_f[0:1, 0:128],
                             rhs=rows[0:1, 3 * ntok + 512 * c:3 * ntok + 512 * c + 512],
                             start=True, stop=True)
            nc.scalar.copy(bc[:, 512 * c:512 * c + 512], pbt[:, 0:512])
            nc.scalar.copy(bc[:, ntok + 512 * c:ntok + 512 * c + 512],
                           pbt[:, 512:1024])
        free_rows()
        for i in range(8):
            t1 = tmpp.tile([P, ntok], BF16, tag="t1", name="t1")
            nc.vector.tensor_mul(t1[:], get_src(i), bc[:, 0:ntok])
            nc.vector.tensor_sub(put_dst(i), t1[:], bc[:, ntok:2 * ntok])
        free_bc()

    # stack-ordered allocation (frees are LIFO): long-lived at the bottom
    x2_all, free_x2 = tc.tile([P, 8192], F32, name="x2_all")
    y2_all, free_y2 = tc.tile([P, 8192], BF16, name="y2_all")
    k_all, free_k = tc.tile([P, 8192], BF16, name="k_all")
    q_all, free_q = tc.tile([P, 8192], BF16, name="q_all")
    v_all, free_v = tc.tile([P, 8320], BF16, name="v_all")
    h1, free_h1 = tc.tile([P, 8192], BF16, name="h1")

    # ---- x resident in SBUF during LN1 (feature-major c-chunks) ----
    x_all, free_x = tc.tile([P, 8192], F32, name="x_all")
    for i in range(8):
        nc.sync.dma_start(x_all[:, ts(i, 1024)], d_xT[ts(i, 128), :])

    # ================= phase 1: LN1 + kq + v =================
    with tc.tile_pool(name="pa1", bufs=3, space="PSUM") as pa1, \
         tc.tile_pool(name="psS1", bufs=2, space="PSUM") as psS1:
        layernorm(lambda i: x_all[:, ts(i, 1024)],
                  lambda i: h1[:, ts(i, 1024)], 1024, psS1, pa1, "h1")
        free_x()
        if dbg:
            nc.sync.dma_start(dbg["h1"][:], h1[:])
        # preload the exp ACT table while ACT is idle (off critical path)
        scr, free_scr = tc.tile([1, 8], F32, name="scr")
        act_raw(nc, scr[0:1, 0:1], cfb[0:1, 193:194], AF.Exp)

        # ---- kq projections: per head, psum [128 = k(64)+q(64), 1024t] ----
        wv_all, free_wv = tc.tile([P, 8192], BF16, name="wv_all")
        wkq_v = d_wkq.rearrange("(ct p) o -> p ct o", p=128)
        for g4 in range(4):
            wg = wp.tile([P, 4096], BF16, tag="wg", name="wg")
            wgv = wg[:].rearrange("p (ct o) -> p ct o", o=512)
            for cc in range(4):
                nc.sync.dma_start(wgv[:, 2 * cc:2 * cc + 2, :],
                                  wkq_v[:, 2 * cc:2 * cc + 2, ts(g4, 512)])
            for hl in range(4):
                hh = 4 * g4 + hl
                j, r = hh // 2, (hh % 2) * 64
                pp = pa1.tile([P, 1024], F32, tag="a", name="ppkq")
                for c in range(8):
                    for ch in range(2):
                        nc.tensor.matmul(pp[:, ts(ch, 512)], lhsT=wgv[:, c, ts(hl, 128)],
                                         rhs=h1[:, 1024 * c + 512 * ch:1024 * c + 512 * ch + 512],
                                         start=(c == 0), stop=(c == 7))
                col = 1024 * j
                nc.scalar.activation(k_all[r:r + 64, col:col + 1024], pp[0:64, :],
                                     AF.Identity, bias=kqb[0:64, hh:hh + 1])
                nc.scalar.activation(q_all[r:r + 64, col:col + 1024], pp[64:128, :],
                                     AF.Identity, bias=kqb[64:128, hh:hh + 1])

        # ---- v projection (token-major, fused ones column per head) ----
        wv_v = d_wv.rearrange("(ct p) o -> p ct o", p=128)
        for c in range(8):
            nc.sync.dma_start(wv_all[:, ts(c, 1024)], wv_v[:, c, :])
        v_view = v_all[:].rearrange("p (a c) -> p a c", c=65)
        nc.vector.memset(v_view[:, :, 64:65], 1.0)
        v_hview = v_all[:].rearrange("p (jt h c) -> p jt h c", jt=8, c=65)
        for jt in range(8):
            psv = pa1.tile([P, 1024], F32, tag="a", name="psv")
            for c in range(8):
                lhs = h1[:, 1024 * c + 128 * jt:1024 * c + 128 * jt + 128]
                for half in range(2):
                    nc.tensor.matmul(psv[:, ts(half, 512)], lhsT=lhs,
                                     rhs=wv_all[:, 1024 * c + 512 * half:1024 * c + 512 * half + 512],
                                     start=(c == 0), stop=(c == 7))
            nc.scalar.copy(v_hview[:, jt, :, 0:64],
                           psv[:].rearrange("p (h c) -> p h c", c=64))
        free_wv()
        free_scr()
        free_h1()
    if dbg:
        nc.sync.dma_start(dbg["k"][:], k_all[:])
        nc.sync.dma_start(dbg["q"][:], q_all[:])
        nc.sync.dma_start(dbg["v"][:], v_all[:])

    # ================= phase 2: attention (ch-major) + proj half 0 =========
    # Per block (j, ch): scores pair (row-group concurrent) -> exp (pure ACT
    # stream) -> diag mask (DVE) -> av accumulate [y'(64); Z] in [65,512]
    # psum pairs. y' evicted to y_all (DVE), Z rows to zpair (GpSimd).
    # Normalization (1/Z on DVE via reciprocal_approx_fast, broadcast via
    # K=1 matmuls, multiply on DVE) is deferred by one block so nothing
    # stalls. proj token-half-0 matmuls are emitted between ch=1 blocks to
    # keep the PE busy while ACT chews the exp backlog.
    # per-block y'+Z eviction target: [65,512] f32 slots (8 = 4-deep block
    # parity x 2 heads), y' dims at partitions 0:64, Z at partition 64.
    ybuf, free_y = tc.tile([65, 4096], F32, name="ybuf")
    e_buf, free_e = tc.tile([P, 6144], BF16, name="e_buf")
    e_rot = [0]

    def e_slot():
        i = e_rot[0] % 6
        e_rot[0] += 1
        return e_buf[:, 1024 * i:1024 * i + 1024]

    pw_v = d_pw.rearrange("(ct p) o -> p ct o", p=128)
    proj_wgs = {}

    def load_proj_wg(jg, nm):
        wg = wp.tile([P, 4096], BF16, tag="wg", name=nm)
        wgv = wg[:].rearrange("p (ct o) -> p ct o", o=512)
        for cc in range(4):
            nc.sync.dma_start(wgv[:, 2 * cc:2 * cc + 2, :],
                              pw_v[:, 2 * cc:2 * cc + 2, ts(jg, 512)])
        proj_wgs[jg] = wgv

    def proj_piece(jj, h, pool):
        wgv = proj_wgs[jj // 4]
        pp = pool.tile([P, 512], F32, tag="pj", name="ppj")
        for c in range(8):
            nc.tensor.matmul(pp[:], lhsT=wgv[:, c, ts(jj % 4, 128)],
                             rhs=y2_all[:, 1024 * c + 512 * h:1024 * c + 512 * h + 512],
                             start=(c == 0), stop=(c == 7))
        xr = tmpp.tile([P, 512], F32, tag="xr", name="xr")
        nc.sync.dma_start(xr[:], d_xT[ts(jj, 128), 512 * h:512 * h + 512])
        nc.vector.scalar_tensor_tensor(
            x2_all[:, 1024 * jj + 512 * h:1024 * jj + 512 * h + 512],
            pp[:], pbc[:, jj:jj + 1], xr[:],
            ALU.add, ALU.add)

    # rotating invZ rows (partition 64, matching ybuf's Z row)
    zibuf, free_zibuf = tc.tile([65, 4096], F32, name="zibuf")
    zrot = [0]

    with tc.tile_pool(name="psA", bufs=2, space="PSUM") as psA, \
         tc.tile_pool(name="psB", bufs=3, space="PSUM") as psB, \
         tc.tile_pool(name="pph0", bufs=1, space="PSUM") as pph0:
        pend = []

        def emit_normalize(j, ch, par):
            col = 1024 * j + 512 * ch
            # NB: recip_approx_fast mislowers single-partition APs at a
            # nonzero base (HW-probed); run it over the full [65,1024] view
            # instead — rows 0:64 (1/y', may hit the undefined ±0 edge case)
            # are junk and never read, only the Z row (64) is consumed.
            nc.vector.reciprocal_approx_fast(
                zibuf[:, ts(par, 1024)], ybuf[:, ts(par, 1024)])
            pz = psA.tile([P, 1024], F32, tag="a", name="pz")
            for m2 in range(2):
                nc.tensor.matmul(pz[0:64, ts(m2, 512)],
                                 lhsT=ones_f[64:65, 0:64],
                                 rhs=zibuf[64:65, 1024 * par + 512 * m2:1024 * par + 512 * m2 + 512],
                                 start=True, stop=True)
            for m2 in range(2):
                r = 64 * m2
                nc.vector.tensor_mul(y2_all[r:r + 64, col:col + 512],
                                     pz[0:64, ts(m2, 512)],
                                     ybuf[0:64, ts(2 * par + m2, 512)])

        def emit_block(j, ch):
            ntk = 4 if ch == 0 else 8
            qcol = 1024 * j + 512 * ch
            py = [psB.tile([65, 512], F32, tag="b", name="py") for _ in range(2)]
            for jt in range(ntk):
                pcol = 1024 * j + 128 * jt
                m = jt - 4 * ch
                o = 128 * m if m > 0 else 0
                ps_ = psA.tile([P, 1024], F32, tag="a", name="ps")
                for m2 in range(2):
                    r = 64 * m2
                    nc.tensor.matmul(ps_[:, 512 * m2 + o:512 * m2 + 512],
                                     lhsT=k_all[r:r + 64, pcol:pcol + 128],
                                     rhs=q_all[r:r + 64, qcol + o:qcol + 512],
                                     start=True, stop=True)
                et_t = e_slot()
                et = et_t.rearrange("p (h c) -> p h c", c=512)
                ps_v = ps_[:].rearrange("p (h c) -> p h c", c=512)
                nc.scalar.activation(et[:, :, o:512], ps_v[:, :, o:512],
                                     AF.Exp, bias=zero_c, scale=0.125)
                if m >= 0:
                    nc.vector.tensor_mul(
                        et[:, :, o:o + 128], et[:, :, o:o + 128],
                        masks[:].rearrange("p (h c) -> p h c", c=512)[:, 0:2, 0:128])
                for m2 in range(2):
                    hh = 2 * j + m2
                    nc.tensor.matmul(
                        py[m2][:, o:512],
                        lhsT=v_all[:, 1040 * jt + 65 * hh:1040 * jt + 65 * hh + 65],
                        rhs=et[:, m2, o:512],
                        start=(jt == 0), stop=(jt == ntk - 1))
                # normalize deferred by two blocks, emitted after this
                # block's second av so its pz never stalls the PE queue.
                if jt == 1 and len(pend) >= 2:
                    emit_normalize(*pend.pop(0))
            par = zrot[0] % 4
            zrot[0] += 1
            for m2 in range(2):
                nc.vector.tensor_copy(ybuf[:, ts(2 * par + m2, 512)],
                                      py[m2][:, :])
            pend.append((j, ch, par))

        for j in range(8):
            emit_block(j, 0)
            if j == 6:
                load_proj_wg(0, "wgp")
            if j == 7:
                load_proj_wg(1, "wgp2")
        for j in range(8):
            emit_block(j, 1)
            # proj half-0 piece jj=j-1: all its y2 half-0 inputs are final
            # once the deferred ch=0 normalizes have run (done by block (1,1)).
            if j >= 2:
                proj_piece(j - 2, 0, pph0)
        while pend:
            emit_normalize(*pend.pop(0))
        proj_piece(6, 0, pph0)
        proj_piece(7, 0, pph0)
    free_zibuf()
    free_e()
    free_y()
    if dbg:
        nc.sync.dma_start(dbg["y"][:], y2_all[:])

    # ================= phase 3: LN2-h0 | proj half 1 | LN2-h1 | fc1 | fc2 ===
    free_v()
    free_q()
    free_k()
    if dbg:
        nc.sync.dma_start(dbg["x2"][:], x2_all[:])
    with tc.tile_pool(name="pC", bufs=2, space="PSUM") as pC, \
         tc.tile_pool(name="pD", bufs=2, space="PSUM") as pD, \
         tc.tile_pool(name="psS2", bufs=2, space="PSUM") as psS2:
        h2, free_h2 = tc.tile([P, 8192], BF16, name="h2")
        # LN2 token-half 0 first: its DVE normalize tail hides under the
        # proj half-1 matmuls that follow on the PE queue.
        layernorm(lambda i: x2_all[:, 1024 * i:1024 * i + 512],
                  lambda i: h2[:, 1024 * i:1024 * i + 512],
                  512, psS2, pC, "h2_0")
        load_proj_wg(0, "wgp1b")
        load_proj_wg(1, "wgp2b")
        for jj in range(8):
            proj_piece(jj, 1, pD)
        layernorm(lambda i: x2_all[:, 1024 * i + 512:1024 * i + 1024],
                  lambda i: h2[:, 1024 * i + 512:1024 * i + 1024],
                  512, psS2, pC, "h2_1")

        g_all, free_g = tc.tile([P, 32768], BF16, name="g_all")
        w1_v = d_w1.rearrange("(ct p) o -> p ct o", p=128)
        for og in range(8):
            wg = wp.tile([P, 4096], BF16, tag="wg", name="wg1")
            wgv = wg[:].rearrange("p (ct o) -> p ct o", o=512)
            for cc in range(4):
                nc.sync.dma_start(wgv[:, 2 * cc:2 * cc + 2, :],
                                  w1_v[:, 2 * cc:2 * cc + 2, ts(og, 512)])
            for ol in range(4):
                oo = 4 * og + ol
                pp = pC.tile([P, 1024], F32, tag="a", name="pp1")
                for c in range(8):
                    for ch in range(2):
                        nc.tensor.matmul(pp[:, ts(ch, 512)], lhsT=wgv[:, c, ts(ol, 128)],
                                         rhs=h2[:, 1024 * c + 512 * ch:1024 * c + 512 * ch + 512],
                                         start=(c == 0), stop=(c == 7))
                nc.scalar.activation(g_all[:, ts(oo, 1024)], pp[:],
                                     AF.Gelu, bias=b1c[:, oo:oo + 1])
        if dbg:
            nc.sync.dma_start(dbg["g"][:], g_all[:])

        # ---- fc2 + residual -> out ----
        w2_v = d_w2.rearrange("(kk p) o -> p kk o", p=128)
        for j in range(8):
            wg = wp.tile([P, 4096], BF16, tag="wg", name="wg2")
            wgv = wg[:].rearrange("p (kk o) -> p kk o", o=128)
            for kg in range(4):
                nc.sync.dma_start(wgv[:, 8 * kg:8 * kg + 8, :],
                                  w2_v[:, 8 * kg:8 * kg + 8, ts(j, 128)])
            pp = pC.tile([P, 1024], F32, tag="a", name="pp2")
            for kk in range(32):
                for ch in range(2):
                    nc.tensor.matmul(pp[:, ts(ch, 512)], lhsT=wgv[:, kk, :],
                                     rhs=g_all[:, 1024 * kk + 512 * ch:1024 * kk + 512 * ch + 512],
                                     start=(kk == 0), stop=(kk == 31))
            x3 = outp.tile([P, 1024], F32, tag="x3", name="x3")
            nc.vector.scalar_tensor_tensor(
                x3[:], pp[:], b2c[:, j:j + 1],
                x2_all[:, ts(j, 1024)], ALU.add, ALU.add)
            nc.sync.dma_start(d_out[ts(j, 128), :], x3[:])
        free_g()
    free_h2()
    free_y2()
    free_x2()
    free_mo()
    free_cfb()


# ---------------- host side ----------------

def prep_inputs(inputs):
    """Build the per-core in_maps from the full problem inputs."""
    f32 = np.float32
    bf16 = ml_dtypes.bfloat16
    x = np.asarray(inputs["x"], f32)
    kqv_w = np.asarray(inputs["kqv_w"], f32)
    kqv_b = np.asarray(inputs["kqv_b"], f32)
    proj_w = np.asarray(inputs["proj_w"], f32)
    proj_b = np.asarray(inputs["proj_b"], f32)
    fc1_w = np.asarray(inputs["fc1_w"], f32)
    fc1_b = np.asarray(inputs["fc1_b"], f32)
    fc2_w = np.asarray(inputs["fc2_w"], f32)
    fc2_b = np.asarray(inputs["fc2_b"], f32)

    wT = np.ascontiguousarray(kqv_w.T).reshape(C, H, 192)
    wkq = np.ascontiguousarray(wT[:, :, :128].reshape(C, 2048)).astype(bf16)
    wv = np.ascontiguousarray(wT[:, :, 128:].reshape(C, 1024)).astype(bf16)
    pw = np.ascontiguousarray(proj_w.T).astype(bf16)
    w1 = np.ascontiguousarray(fc1_w.T).astype(bf16)
    w2 = np.ascontiguousarray(fc2_w.T).astype(bf16)

    kq_b = kqv_b.reshape(H, 192)[:, :128].T  # [128, 16]
    v_b = kqv_b.reshape(H, 192)[:, 128:].reshape(C)
    pb = proj_b + proj_w.astype(np.float64) @ v_b.astype(np.float64)
    pb_col = pb.astype(f32).reshape(8, 128).T  # [128, 8]
    b1_col = fc1_b.reshape(32, 128).T  # [128, 32]
    b2_col = fc2_b.reshape(8, 128).T  # [128, 8]

    cfb = np.zeros((P, 200), f32)
    cfb[:, 0:128] = 1.0
    cfb[:, 128:144] = kq_b
    cfb[:, 144:152] = pb_col
    cfb[:, 152:184] = b1_col
    cfb[:, 184:192] = b2_col
    cfb[:, 192] = LN_EPS

    mo = np.zeros((P, 1024), np.float32)
    pcol = np.arange(128)[:, None]
    frow = np.arange(512)[None, :]
    blk = (frow >= pcol).astype(np.float32)
    mo[:, 0:512] = blk
    mo[:, 512:1024] = blk
    mo = mo.astype(bf16)

    xT = np.ascontiguousarray(x.transpose(0, 2, 1)).astype(f32)  # [B, C, T]

    shared = dict(wkq=wkq, wv=wv, pw=pw, w1=w1, w2=w2, cfb=cfb, mo=mo)
    in_maps = [dict(shared, xT=xT[b]) for b in range(NB)]
    return in_maps


_CACHE = {}


def get_nc(debug=False):
    key = bool(debug)
    if key not in _CACHE:
        _CACHE[key] = build_nc(debug=debug)
    return _CACHE[key]


def run(inputs, debug=False, trace=False):
    nc = get_nc(debug=debug)
    in_maps = prep_inputs(inputs)
    res = bass_utils.run_bass_kernel_spmd(nc, in_maps, core_ids=list(range(NB)),
                                          trace=trace)
    return res


def kernel(**inputs):
    res = run(inputs, debug=False, trace=False)
    out = np.stack([np.asarray(res.results[b]["out"]).T for b in range(NB)])
    return np.ascontiguousarray(out.astype(np.float32))


# revision 18
# speedup vs baseline: 1.1229x; 1.0977x over previous
"""Trainium2 Bass kernel for nn_Block_38053410242840 (dense transformer block).

Strategy: data-parallel over batch (B=8 -> 8 NeuronCores, zero collectives).
Per core, one batch element [T=1024, C=1024] flows feature-major
(activations stored [feature partitions, token free]) so every matmul's
contraction dim sits on SBUF partitions with no on-device transposes:
the host pre-transposes weights/x and pre-casts weights to bf16.

v2 pipeline (vs v1): x held resident in SBUF; LN uses a single Rsqrt and a
bf16 (rstd | mean*rstd) broadcast; attention runs ch-major (all heads'
first token half, then second) with softmax normalization deferred by one
block and 1/Z computed on DVE (reciprocal_approx_fast) so the ACT engine
runs a pure exp stream with no table reloads; proj's first token half is
emitted between attention blocks to fill the ACT-bound PE gaps; LN2/fc1
run per token half so fc1 overlaps LN2; PSUM pools are scoped per phase.
"""
import sys

sys.path.insert(0, "/opt/trn_rl_repo")

from contextlib import ExitStack

import ml_dtypes
import numpy as np

import concourse.bass as bass
import concourse.tile as tile
from concourse import bacc, mybir
from concourse import bass_utils

F32 = mybir.dt.float32
BF16 = mybir.dt.bfloat16
AF = mybir.ActivationFunctionType
ALU = mybir.AluOpType
ts = bass.ts

P = 128
T = 1024
C = 1024
H = 16
HD = 64
LN_EPS = 1e-5
NB = 8  # cores / batch


def act_raw(nc, out, in_, func, bias=0.0, scale=1.0):
    """InstActivation with immediate bias/scale (bypasses accuracy guards)."""
    eng = nc.scalar
    inputs = [eng.lower_ap(in_)]
    for arg in (bias, scale, 0.0):
        inputs.append(mybir.ImmediateValue(dtype=mybir.dt.float32, value=arg))
    return eng.add_instruction(
        mybir.InstActivation(
            name=nc.get_next_instruction_name(),
            func=func,
            ins=inputs,
            outs=[eng.lower_ap(out)],
        )
    )


def build_nc(debug=False):
    nc = bacc.Bacc("TRN2", target_bir_lowering=False, debug=False,
                   enable_asserts=False, num_devices=NB)

    d_xT = nc.dram_tensor("xT", [C, T], F32, kind="ExternalInput").ap()
    d_wkq = nc.dram_tensor("wkq", [C, 2048], BF16, kind="ExternalInput").ap()
    d_wv = nc.dram_tensor("wv", [C, 1024], BF16, kind="ExternalInput").ap()
    d_pw = nc.dram_tensor("pw", [C, 1024], BF16, kind="ExternalInput").ap()
    d_w1 = nc.dram_tensor("w1", [C, 4096], BF16, kind="ExternalInput").ap()
    d_w2 = nc.dram_tensor("w2", [4096, 1024], BF16, kind="ExternalInput").ap()
    # packed f32 consts: [:,0:128]=ones, 128:144 kq bias, 144:152 proj bias(+pb),
    # 152:184 fc1 bias, 184:192 fc2 bias
    d_cfb = nc.dram_tensor("cfb", [P, 336], F32, kind="ExternalInput").ap()
    # packed bf16 consts: [:,0:1024]=causal diag-block masks (2x512), col 1023 ones
    d_mo = nc.dram_tensor("mo", [P, 1024], BF16, kind="ExternalInput").ap()
    d_out = nc.dram_tensor("out", [C, T], F32, kind="ExternalOutput").ap()

    dbg = {}
    if debug:
        dbg["h1"] = nc.dram_tensor("dbg_h1", [P, 8192], BF16, kind="ExternalOutput").ap()
        dbg["k"] = nc.dram_tensor("dbg_k", [P, 8192], BF16, kind="ExternalOutput").ap()
        dbg["q"] = nc.dram_tensor("dbg_q", [P, 8192], BF16, kind="ExternalOutput").ap()
        dbg["v"] = nc.dram_tensor("dbg_v", [P, 8320], BF16, kind="ExternalOutput").ap()
        dbg["y"] = nc.dram_tensor("dbg_y", [P, 8192], BF16, kind="ExternalOutput").ap()
        dbg["x2"] = nc.dram_tensor("dbg_x2", [P, 8192], F32, kind="ExternalOutput").ap()
        dbg["g"] = nc.dram_tensor("dbg_g", [P, 32768], BF16, kind="ExternalOutput").ap()

    with tile.TileContext(nc) as tc:
        with ExitStack() as ctx:
            build_body(ctx, tc, nc, d_xT, d_wkq, d_wv, d_pw, d_w1, d_w2,
                       d_cfb, d_mo, d_out, dbg)
    nc.compile()
    return nc


def build_body(ctx, tc, nc, d_xT, d_wkq, d_wv, d_pw, d_w1, d_w2, d_cfb, d_mo,
               d_out, dbg):
    wp = ctx.enter_context(tc.tile_pool(name="wp", bufs=2))
    lnp = ctx.enter_context(tc.tile_pool(name="lnp", bufs=2))
    tmpp = ctx.enter_context(tc.tile_pool(name="tmpp", bufs=2))
    outp = ctx.enter_context(tc.tile_pool(name="outp", bufs=2))

    # ---- constants ----
    cfb, free_cfb = tc.tile([P, 336], F32, name="cfb_t")
    nc.sync.dma_start(cfb[:], d_cfb[:])
    mo, free_mo = tc.tile([P, 1024], BF16, name="mo_t")
    nc.sync.dma_start(mo[:], d_mo[:])
    ones_f = cfb[:, 0:128]
    kqb = cfb[:, 128:144]
    pbc = cfb[:, 144:152]
    b1c = cfb[:, 152:184]
    b2c = cfb[:, 184:192]
    zero_c = cfb[:, 193:194]
    sel2 = cfb[:, 200:328]  # rows 64/65: head-pair invZ selector for K=2 bcast
    masks = mo[:, 0:1024]  # diagonal-block mask, doubled
    ones_b = mo[:, 1023:1024]  # all-ones column (bf16)


    class LN:
        """Incremental LN emitter: chunk(i) after each source chunk is ready,
        finish() computes rows/broadcast, normalize() writes the bf16 dest."""

        def __init__(self, get_src, put_dst, ntok, psS_pool, pb_pool, hname):
            self.get_src, self.put_dst = get_src, put_dst
            self.ntok, self.ncol, self.hname = ntok, ntok // 512, hname
            self.pb_pool = pb_pool
            self.psS = [psS_pool.tile([33, 512], F32, tag="s", name=f"psS_{hname}")
                        for _ in range(self.ncol)]

        def chunk(self, i):
            ntok = self.ntok
            xbsq = lnp.tile([P, 2 * ntok], BF16, tag="xbsq", name="xbsq")
            xb = xbsq[:, 0:ntok]
            sq = xbsq[:, ntok:2 * ntok]
            nc.scalar.copy(xb, self.get_src(i))
            nc.vector.tensor_mul(sq, xb, xb)
            for c in range(self.ncol):
                nc.tensor.matmul(self.psS[c][0:1, :], lhsT=ones_b[:, 0:1],
                                 rhs=xb[:, ts(c, 512)], start=(i == 0), stop=(i == 7))
                nc.tensor.matmul(self.psS[c][32:33, :], lhsT=ones_b[:, 0:1],
                                 rhs=sq[:, ts(c, 512)], start=(i == 0), stop=(i == 7),
                                 tile_position=(0, 32))

        def finish(self):
            # stats rows, all on partition 0 (DVE two-SBUF-input ops require
            # equal base partitions): mean | E[x2] -> var | rstd | mean*rstd
            ntok, hname = self.ntok, self.hname
            self.bc, self.free_bc = tc.tile([P, 2 * ntok], BF16, name=hname + "_bc")
            bc = self.bc
            rows, free_rows = tc.tile([1, 4 * ntok], F32, name=hname + "_rows")
            for c in range(self.ncol):
                nc.vector.tensor_scalar_mul(rows[0:1, ts(c, 512)],
                                            self.psS[c][0:1, :], 1.0 / C)
                nc.vector.tensor_scalar_mul(rows[0:1, ntok + 512 * c:ntok + 512 * c + 512],
                                            self.psS[c][32:33, :], 1.0 / C)
            nc.vector.tensor_mul(rows[0:1, 2 * ntok:3 * ntok], rows[0:1, 0:ntok],
                                 rows[0:1, 0:ntok])
            nc.vector.tensor_sub(rows[0:1, ntok:2 * ntok], rows[0:1, ntok:2 * ntok],
                                 rows[0:1, 2 * ntok:3 * ntok])
            act_raw(nc, rows[0:1, 2 * ntok:3 * ntok], rows[0:1, ntok:2 * ntok],
                    AF.Rsqrt, bias=LN_EPS)
            nc.vector.tensor_mul(rows[0:1, 3 * ntok:4 * ntok], rows[0:1, 0:ntok],
                                 rows[0:1, 2 * ntok:3 * ntok])
            for c in range(self.ncol):
                pbt = self.pb_pool.tile([P, 1024], F32, tag="a", name=f"pb_{hname}")
                nc.tensor.matmul(pbt[:, 0:512], lhsT=ones_f[0:1, 0:128],
                                 rhs=rows[0:1, 2 * ntok + 512 * c:2 * ntok + 512 * c + 512],
                                 start=True, stop=True)
                nc.tensor.matmul(pbt[:, 512:1024], lhsT=ones_f[0:1, 0:128],
                                 rhs=rows[0:1, 3 * ntok + 512 * c:3 * ntok + 512 * c + 512],
                                 start=True, stop=True)
                nc.scalar.copy(bc[:, 512 * c:512 * c + 512], pbt[:, 0:512])
                nc.scalar.copy(bc[:, ntok + 512 * c:ntok + 512 * c + 512],
                               pbt[:, 512:1024])
            free_rows()

        def normalize(self):
            ntok, bc = self.ntok, self.bc
            for i in range(8):
                t1 = tmpp.tile([P, ntok], BF16, tag="t1", name="t1")
                nc.vector.tensor_mul(t1[:], self.get_src(i), bc[:, 0:ntok])
                nc.vector.tensor_sub(self.put_dst(i), t1[:], bc[:, ntok:2 * ntok])

    def layernorm(get_src, put_dst, ntok, psS_pool, pb_pool, hname):
        ln = LN(get_src, put_dst, ntok, psS_pool, pb_pool, hname)
        for i in range(8):
            ln.chunk(i)
        ln.finish()
        ln.normalize()
        ln.free_bc()

    # stack-ordered allocation (frees are LIFO): long-lived at the bottom
    x2_all, free_x2 = tc.tile([P, 8192], F32, name="x2_all")
    y2_all, free_y2 = tc.tile([P, 8192], BF16, name="y2_all")
    k_all, free_k = tc.tile([P, 8192], BF16, name="k_all")
    q_all, free_q = tc.tile([P, 8192], BF16, name="q_all")
    v_all, free_v = tc.tile([P, 8320], BF16, name="v_all")
    h1, free_h1 = tc.tile([P, 8192], BF16, name="h1")

    # ---- x resident in SBUF during LN1 (feature-major c-chunks) ----
    x_all, free_x = tc.tile([P, 8192], F32, name="x_all")
    for i in range(8):
        nc.sync.dma_start(x_all[:, ts(i, 1024)], d_xT[ts(i, 128), :])

    # ================= phase 1: LN1 + kq + v =================
    with tc.tile_pool(name="pa1", bufs=3, space="PSUM") as pa1, \
         tc.tile_pool(name="psS1", bufs=2, space="PSUM") as psS1:
        layernorm(lambda i: x_all[:, ts(i, 1024)],
                  lambda i: h1[:, ts(i, 1024)], 1024, psS1, pa1, "h1")
        free_x()
        if dbg:
            nc.sync.dma_start(dbg["h1"][:], h1[:])
        # preload the exp ACT table while ACT is idle (off critical path)
        scr, free_scr = tc.tile([1, 8], F32, name="scr")
        act_raw(nc, scr[0:1, 0:1], cfb[0:1, 193:194], AF.Exp)

        # ---- kq projections: per head, psum [128 = k(64)+q(64), 1024t] ----
        wv_all, free_wv = tc.tile([P, 8192], BF16, name="wv_all")
        wkq_v = d_wkq.rearrange("(ct p) o -> p ct o", p=128)
        for g4 in range(4):
            wg = wp.tile([P, 4096], BF16, tag="wg", name="wg")
            wgv = wg[:].rearrange("p (ct o) -> p ct o", o=512)
            for cc in range(4):
                nc.sync.dma_start(wgv[:, 2 * cc:2 * cc + 2, :],
                                  wkq_v[:, 2 * cc:2 * cc + 2, ts(g4, 512)])
            for hl in range(4):
                hh = 4 * g4 + hl
                j, r = hh // 2, (hh % 2) * 64
                pp = pa1.tile([P, 1024], F32, tag="a", name="ppkq")
                for c in range(8):
                    for ch in range(2):
                        nc.tensor.matmul(pp[:, ts(ch, 512)], lhsT=wgv[:, c, ts(hl, 128)],
                                         rhs=h1[:, 1024 * c + 512 * ch:1024 * c + 512 * ch + 512],
                                         start=(c == 0), stop=(c == 7))
                col = 1024 * j
                nc.scalar.activation(k_all[r:r + 64, col:col + 1024], pp[0:64, :],
                                     AF.Identity, bias=kqb[0:64, hh:hh + 1])
                nc.scalar.activation(q_all[r:r + 64, col:col + 1024], pp[64:128, :],
                                     AF.Identity, bias=kqb[64:128, hh:hh + 1])

        # ---- v projection (token-major, fused ones column per head) ----
        wv_v = d_wv.rearrange("(ct p) o -> p ct o", p=128)
        for c in range(8):
            nc.sync.dma_start(wv_all[:, ts(c, 1024)], wv_v[:, c, :])
        v_view = v_all[:].rearrange("p (a c) -> p a c", c=65)
        nc.vector.memset(v_view[:, :, 64:65], 1.0)
        v_hview = v_all[:].rearrange("p (jt h c) -> p jt h c", jt=8, c=65)
        for jt in range(8):
            psv = pa1.tile([P, 1024], F32, tag="a", name="psv")
            for c in range(8):
                lhs = h1[:, 1024 * c + 128 * jt:1024 * c + 128 * jt + 128]
                for half in range(2):
                    nc.tensor.matmul(psv[:, ts(half, 512)], lhsT=lhs,
                                     rhs=wv_all[:, 1024 * c + 512 * half:1024 * c + 512 * half + 512],
                                     start=(c == 0), stop=(c == 7))
            nc.scalar.copy(v_hview[:, jt, :, 0:64],
                           psv[:].rearrange("p (h c) -> p h c", c=64))
        free_wv()
        free_scr()
        free_h1()
    if dbg:
        nc.sync.dma_start(dbg["k"][:], k_all[:])
        nc.sync.dma_start(dbg["q"][:], q_all[:])
        nc.sync.dma_start(dbg["v"][:], v_all[:])

    # ================= phase 2: attention (ch-major) + proj half 0 =========
    # Per block (j, ch): scores pair (row-group concurrent) -> exp (pure ACT
    # stream) -> diag mask (DVE) -> av accumulate [y'(64); Z] in [65,512]
    # psum pairs. y' evicted to y_all (DVE), Z rows to zpair (GpSimd).
    # Normalization (1/Z on DVE via reciprocal_approx_fast, broadcast via
    # K=1 matmuls, multiply on DVE) is deferred by one block so nothing
    # stalls. proj token-half-0 matmuls are emitted between ch=1 blocks to
    # keep the PE busy while ACT chews the exp backlog.
    # per-block eviction targets, 4-deep block parity: y' head pair packed
    # [128,512] bf16; Z rows f32 at partitions 64 (head0) / 65 (head1).
    ybuf, free_y = tc.tile([P, 2048], BF16, name="ybuf")
    zrows, free_zr = tc.tile([97, 2048], F32, name="zrows")
    nc.vector.memset(zrows[:], 1.0)
    e_buf, free_e = tc.tile([P, 6144], BF16, name="e_buf")
    e_rot = [0]

    def e_slot():
        i = e_rot[0] % 6
        e_rot[0] += 1
        return e_buf[:, 1024 * i:1024 * i + 1024]

    pw_v = d_pw.rearrange("(ct p) o -> p ct o", p=128)
    proj_wgs = {}

    def load_proj_wg(jg, nm):
        wg = wp.tile([P, 4096], BF16, tag="wg", name=nm)
        wgv = wg[:].rearrange("p (ct o) -> p ct o", o=512)
        for cc in range(4):
            nc.sync.dma_start(wgv[:, 2 * cc:2 * cc + 2, :],
                              pw_v[:, 2 * cc:2 * cc + 2, ts(jg, 512)])
        proj_wgs[jg] = wgv

    def proj_piece(jj, h, pool):
        wgv = proj_wgs[jj // 4]
        pp = pool.tile([P, 512], F32, tag="pj", name="ppj")
        for c in range(8):
            nc.tensor.matmul(pp[:], lhsT=wgv[:, c, ts(jj % 4, 128)],
                             rhs=y2_all[:, 1024 * c + 512 * h:1024 * c + 512 * h + 512],
                             start=(c == 0), stop=(c == 7))
        xr = tmpp.tile([P, 512], F32, tag="xr", name="xr")
        nc.sync.dma_start(xr[:], d_xT[ts(jj, 128), 512 * h:512 * h + 512])
        nc.vector.scalar_tensor_tensor(
            x2_all[:, 1024 * jj + 512 * h:1024 * jj + 512 * h + 512],
            pp[:], pbc[:, jj:jj + 1], xr[:],
            ALU.add, ALU.add)

    # rotating invZ rows (partitions 64:66, matching zrows)
    zibuf, free_zibuf = tc.tile([97, 2048], F32, name="zibuf")
    zrot = [0]

    with tc.tile_pool(name="psA", bufs=2, space="PSUM") as psA, \
         tc.tile_pool(name="psB", bufs=3, space="PSUM") as psB, \
         tc.tile_pool(name="pph0", bufs=1, space="PSUM") as pph0:
        pend = []

        def emit_normalize(j, ch, par):
            col = 1024 * j + 512 * ch
            # NB: recip_approx_fast mislowers partition-base!=0 APs
            # (HW-probed); run it over the full [66,512] view from base 0 —
            # rows 0:64 are memset-1.0 padding, only rows 64:66 (Z) are real.
            nc.vector.reciprocal_approx_fast(
                zibuf[:, ts(par, 512)], zrows[:, ts(par, 512)])
            pz = psA.tile([P, 1024], F32, tag="a", name="pz")
            nc.tensor.matmul(pz[:, 0:512], lhsT=sel2[64:97, :],
                             rhs=zibuf[64:97, ts(par, 512)], start=True, stop=True)
            nc.vector.tensor_mul(y2_all[:, col:col + 512], pz[:, 0:512],
                                 ybuf[:, ts(par, 512)])

        def emit_block(j, ch):
            ntk = 4 if ch == 0 else 8
            qcol = 1024 * j + 512 * ch
            py = [psB.tile([65, 512], F32, tag="b", name="py") for _ in range(2)]
            for jt in range(ntk):
                pcol = 1024 * j + 128 * jt
                m = jt - 4 * ch
                o = 128 * m if m > 0 else 0
                ps_ = psA.tile([P, 1024], F32, tag="a", name="ps")
                for m2 in range(2):
                    r = 64 * m2
                    nc.tensor.matmul(ps_[:, 512 * m2 + o:512 * m2 + 512],
                                     lhsT=k_all[r:r + 64, pcol:pcol + 128],
                                     rhs=q_all[r:r + 64, qcol + o:qcol + 512],
                                     start=True, stop=True)
                et_t = e_slot()
                et = et_t.rearrange("p (h c) -> p h c", c=512)
                ps_v = ps_[:].rearrange("p (h c) -> p h c", c=512)
                nc.scalar.activation(et[:, :, o:512], ps_v[:, :, o:512],
                                     AF.Exp, bias=zero_c, scale=0.125)
                if m >= 0:
                    nc.vector.tensor_mul(
                        et[:, :, o:o + 128], et[:, :, o:o + 128],
                        masks[:].rearrange("p (h c) -> p h c", c=512)[:, 0:2, 0:128])
                for m2 in range(2):
                    hh = 2 * j + m2
                    nc.tensor.matmul(
                        py[m2][:, o:512],
                        lhsT=v_all[:, 1040 * jt + 65 * hh:1040 * jt + 65 * hh + 65],
                        rhs=et[:, m2, o:512],
                        start=(jt == 0), stop=(jt == ntk - 1))
                # normalize deferred by two blocks, emitted after this
                # block's second av so its pz never stalls the PE queue.
                if jt == 1 and len(pend) >= 2:
                    emit_normalize(*pend.pop(0))
            par = zrot[0] % 4
            zrot[0] += 1
            for m2 in range(2):
                nc.vector.tensor_copy(ybuf[64 * m2:64 * m2 + 64, ts(par, 512)],
                                      py[m2][0:64, :])
                nc.vector.tensor_copy(zrows[64 + 32 * m2:65 + 32 * m2, ts(par, 512)],
                                      py[m2][64:65, :])
            pend.append((j, ch, par))

        for j in range(8):
            emit_block(j, 0)
            if j == 6:
                load_proj_wg(0, "wgp")
            if j == 7:
                load_proj_wg(1, "wgp2")
        for j in range(8):
            emit_block(j, 1)
            # proj half-0 piece jj=j-1: all its y2 half-0 inputs are final
            # once the deferred ch=0 normalizes have run (done by block (1,1)).
            if j >= 2:
                proj_piece(j - 2, 0, pph0)
        while pend:
            emit_normalize(*pend.pop(0))
        proj_piece(6, 0, pph0)
        proj_piece(7, 0, pph0)
    free_zibuf()
    free_e()
    free_zr()
    free_y()
    if dbg:
        nc.sync.dma_start(dbg["y"][:], y2_all[:])

    # ================= phase 3: LN2-h0 | proj half 1 | LN2-h1 | fc1 | fc2 ===
    free_v()
    free_q()
    free_k()
    if dbg:
        nc.sync.dma_start(dbg["x2"][:], x2_all[:])
    with tc.tile_pool(name="pC", bufs=2, space="PSUM") as pC, \
         tc.tile_pool(name="pD", bufs=2, space="PSUM") as pD, \
         tc.tile_pool(name="psS2", bufs=2, space="PSUM") as psS2:
        h2, free_h2 = tc.tile([P, 8192], BF16, name="h2")
        # LN2 half 0: its sources (x2 half 0) completed during attention.
        ln0 = LN(lambda i: x2_all[:, 1024 * i:1024 * i + 512],
                 lambda i: h2[:, 1024 * i:1024 * i + 512],
                 512, psS2, pC, "h2_0")
        for i in range(8):
            ln0.chunk(i)
        ln0.finish()
        # proj half 1 interleaved with LN2 half 1 stats (chunk jj ready
        # right after piece jj's residual add).
        ln1 = LN(lambda i: x2_all[:, 1024 * i + 512:1024 * i + 1024],
                 lambda i: h2[:, 1024 * i + 512:1024 * i + 1024],
                 512, psS2, pC, "h2_1")
        load_proj_wg(0, "wgp1b")
        load_proj_wg(1, "wgp2b")
        for jj in range(8):
            proj_piece(jj, 1, pD)
            ln1.chunk(jj)
        ln1.finish()
        ln0.normalize()

        g_all, free_g = tc.tile([P, 32768], BF16, name="g_all")
        w1_v = d_w1.rearrange("(ct p) o -> p ct o", p=128)

        def fc1_half(h):
            for og in range(8):
                wg = wp.tile([P, 4096], BF16, tag="wg", name="wg1")
                wgv = wg[:].rearrange("p (ct o) -> p ct o", o=512)
                for cc in range(4):
                    nc.sync.dma_start(wgv[:, 2 * cc:2 * cc + 2, :],
                                      w1_v[:, 2 * cc:2 * cc + 2, ts(og, 512)])
                for ol in range(4):
                    oo = 4 * og + ol
                    pp = pD.tile([P, 512], F32, tag="pj", name="pp1")
                    for c in range(8):
                        nc.tensor.matmul(pp[:], lhsT=wgv[:, c, ts(ol, 128)],
                                         rhs=h2[:, 1024 * c + 512 * h:1024 * c + 512 * h + 512],
                                         start=(c == 0), stop=(c == 7))
                    nc.scalar.activation(
                        g_all[:, 1024 * oo + 512 * h:1024 * oo + 512 * h + 512],
                        pp[:], AF.Gelu, bias=b1c[:, oo:oo + 1])

        fc1_half(0)
        ln1.normalize()
        fc1_half(1)
        if dbg:
            nc.sync.dma_start(dbg["g"][:], g_all[:])

        # ---- fc2 + residual -> out ----
        w2_v = d_w2.rearrange("(kk p) o -> p kk o", p=128)
        for j in range(8):
            wg = wp.tile([P, 4096], BF16, tag="wg", name="wg2")
            wgv = wg[:].rearrange("p (kk o) -> p kk o", o=128)
            for kg in range(4):
                nc.sync.dma_start(wgv[:, 8 * kg:8 * kg + 8, :],
                                  w2_v[:, 8 * kg:8 * kg + 8, ts(j, 128)])
            pp = pC.tile([P, 1024], F32, tag="a", name="pp2")
            for kk in range(32):
                for ch in range(2):
                    nc.tensor.matmul(pp[:, ts(ch, 512)], lhsT=wgv[:, kk, :],
                                     rhs=g_all[:, 1024 * kk + 512 * ch:1024 * kk + 512 * ch + 512],
                                     start=(kk == 0), stop=(kk == 31))
            x3 = outp.tile([P, 1024], F32, tag="x3", name="x3")
            nc.vector.scalar_tensor_tensor(
                x3[:], pp[:], b2c[:, j:j + 1],
                x2_all[:, ts(j, 1024)], ALU.add, ALU.add)
            nc.sync.dma_start(d_out[ts(j, 128), :], x3[:])
        free_g()
        ln1.free_bc()
        ln0.free_bc()
    free_h2()
    free_y2()
    free_x2()
    free_mo()
    free_cfb()


# ---------------- host side ----------------

def prep_inputs(inputs):
    """Build the per-core in_maps from the full problem inputs."""
    f32 = np.float32
    bf16 = ml_dtypes.bfloat16
    x = np.asarray(inputs["x"], f32)
    kqv_w = np.asarray(inputs["kqv_w"], f32)
    kqv_b = np.asarray(inputs["kqv_b"], f32)
    proj_w = np.asarray(inputs["proj_w"], f32)
    proj_b = np.asarray(inputs["proj_b"], f32)
    fc1_w = np.asarray(inputs["fc1_w"], f32)
    fc1_b = np.asarray(inputs["fc1_b"], f32)
    fc2_w = np.asarray(inputs["fc2_w"], f32)
    fc2_b = np.asarray(inputs["fc2_b"], f32)

    wT = np.ascontiguousarray(kqv_w.T).reshape(C, H, 192)
    wkq = np.ascontiguousarray(wT[:, :, :128].reshape(C, 2048)).astype(bf16)
    wv = np.ascontiguousarray(wT[:, :, 128:].reshape(C, 1024)).astype(bf16)
    pw = np.ascontiguousarray(proj_w.T).astype(bf16)
    w1 = np.ascontiguousarray(fc1_w.T).astype(bf16)
    w2 = np.ascontiguousarray(fc2_w.T).astype(bf16)

    kq_b = kqv_b.reshape(H, 192)[:, :128].T  # [128, 16]
    v_b = kqv_b.reshape(H, 192)[:, 128:].reshape(C)
    pb = proj_b + proj_w.astype(np.float64) @ v_b.astype(np.float64)
    pb_col = pb.astype(f32).reshape(8, 128).T  # [128, 8]
    b1_col = fc1_b.reshape(32, 128).T  # [128, 32]
    b2_col = fc2_b.reshape(8, 128).T  # [128, 8]

    cfb = np.zeros((P, 336), f32)
    cfb[:, 0:128] = 1.0
    cfb[:, 128:144] = kq_b
    cfb[:, 144:152] = pb_col
    cfb[:, 152:184] = b1_col
    cfb[:, 184:192] = b2_col
    cfb[:, 192] = LN_EPS
    cfb[64, 200:264] = 1.0  # selector: head0 invZ -> out partitions 0:64
    cfb[96, 264:328] = 1.0  # selector: head1 invZ -> out partitions 64:128

    mo = np.zeros((P, 1024), np.float32)
    pcol = np.arange(128)[:, None]
    frow = np.arange(512)[None, :]
    blk = (frow >= pcol).astype(np.float32)
    mo[:, 0:512] = blk
    mo[:, 512:1024] = blk
    mo = mo.astype(bf16)

    xT = np.ascontiguousarray(x.transpose(0, 2, 1)).astype(f32)  # [B, C, T]

    shared = dict(wkq=wkq, wv=wv, pw=pw, w1=w1, w2=w2, cfb=cfb, mo=mo)
    in_maps = [dict(shared, xT=xT[b]) for b in range(NB)]
    return in_maps


_CACHE = {}


def get_nc(debug=False):
    key = bool(debug)
    if key not in _CACHE:
        _CACHE[key] = build_nc(debug=debug)
    return _CACHE[key]


def run(inputs, debug=False, trace=False):
    nc = get_nc(debug=debug)
    in_maps = prep_inputs(inputs)
    res = bass_utils.run_bass_kernel_spmd(nc, in_maps, core_ids=list(range(NB)),
                                          trace=trace)
    return res


def kernel(**inputs):
    res = run(inputs, debug=False, trace=False)
    out = np.stack([np.asarray(res.results[b]["out"]).T for b in range(NB)])
    return np.ascontiguousarray(out.astype(np.float32))
